# revision 19
# baseline (speedup 1.0000x reference)
"""KoLeo loss kernel for Trainium2 (8 NeuronCores, Bass/Tile).

reference semantics:
    x = student_output / max(||row||_2, 1e-8)        # [B, D] row-normalize
    dots = x @ x.T ; dots[i,i] = -1
    nn = argmax(dots, axis=1)
    d_i = || x_i - x_nn(i) + 1e-8 ||_2
    loss = mean(-log(d_i + 1e-8))

Device strategy (data-parallel over rows, 8 cores, identical NEFF):
  * Host normalizes rows in f32, scales by S=64 and quantizes to fp8 e4m3
    (TRN float8e4).  Each core receives the full quantized matrix in a
    transposed, chunk-contiguous layout plus its own 1024-row slice.
  * Device computes the row-sharded Gram G = q_local @ q_all.T with
    DoubleRow fp8 matmuls (K=256 per instruction, 2x PE throughput).
    Column tiles are processed in groups (two ramp groups of 1024
    columns while the input DMAs stream, then 2048-column groups),
    each accumulating into one multi-bank PSUM tile.
  * The Scalar (ACT) engine drains PSUM to SBUF in 1024-wide copies
    (the first half overlaps the group's remaining matmuls); DVE folds
    the group columnwise with a max tree down to [128, 512] (the row's
    true NN always survives: it is the row maximum, so it wins its
    column slot; GpSimd has no max ALU), then extracts the top-8 values
    + column indices (max / max_index).
  * Input DMAs are spread across the Sync/GpSimd/Scalar rings in
    consumption order; a short burst of warmup matmuls on a const tile
    flips the PE HAM clock gate to 8/8 while the first chunks stream.
  * Host merges the per-group needles, takes the global top-8 by device
    value per row (the true NN is always the top-1 needle), expands the
    per-group column ambiguity, recomputes exact f32 cosines for those
    candidates, drops the self-match, picks the true argmax and
    evaluates the reference loss formula exactly.
"""

import numpy as np
import ml_dtypes

import concourse.bacc as bacc
import concourse.bass as bass
import concourse.mybir as mybir
import concourse.tile as tile
from concourse import bass_utils

B, D, P = 8192, 1024, 128
NCORES = 8
LOCAL = B // NCORES  # 1024 rows per core
KT = D // P          # 8 contraction tiles of 128
MT = LOCAL // P      # 8 local row tiles
NJ = 512             # moving free dim per matmul
JT = B // NJ         # 16 column tiles
CH = 16              # input column chunks (one per column tile)
CB = B // CH         # 512 columns per chunk
GROUPS = [(0, 2), (2, 2), (4, 4), (8, 4), (12, 4)]  # (first chunk, #chunks)
NG = len(GROUPS)
S = 64.0             # fp8 pre-scale for normalized rows
EPS = 1e-8

F32 = mybir.dt.float32
FP8 = mybir.dt.float8e4
U16 = mybir.dt.uint16
DR = mybir.MatmulPerfMode.DoubleRow


def emit_kernel(tc, x_ap, xl_ap, cv_ap, ci_ap):
    nc = tc.nc
    with (
        tc.tile_pool(name="big", bufs=1) as big,
        tc.tile_pool(name="work", bufs=3) as work,
        tc.tile_pool(name="ps", bufs=2, space="PSUM") as pp,
    ):
        xT = big.tile([P, CH, KT, CB], FP8)
        xTl = big.tile([P, MT, KT, P], FP8)
        cv = big.tile([P, NG, MT, 8], F32)
        ci = big.tile([P, NG, MT, 8], U16)

        # input DMAs in consumption order, spread over the three DMA
        # rings; the Scalar ring gets only two early chunks so ACT
        # drains never queue behind DMA issues
        def ldx(c, eng):
            eng.dma_start(out=xT[:, c], in_=x_ap[:, c])

        nc.sync.dma_start(out=xTl[:, 0], in_=xl_ap[:, 0])
        ldx(0, nc.scalar)
        ldx(1, nc.gpsimd)
        ldx(2, nc.sync)
        nc.sync.dma_start(out=xTl[:, 1], in_=xl_ap[:, 1])
        nc.sync.dma_start(out=xTl[:, 2], in_=xl_ap[:, 2])
        ldx(3, nc.scalar)
        ldx(4, nc.gpsimd)
        ldx(5, nc.sync)
        for mt in range(3, MT):
            nc.sync.dma_start(out=xTl[:, mt], in_=xl_ap[:, mt])
        for c in range(6, CH):
            ldx(c, nc.gpsimd if c % 2 == 0 else nc.sync)

        # PE warmup on a resident const tile while the first chunks
        # stream, so the HAM clock gate is at 8/8 for the real matmuls
        wsrc = big.tile([P, 2, 256], FP8)
        nc.vector.memset(wsrc[:], 0.25)
        wps = pp.tile([P, 4 * NJ], F32, tag="ps")  # recycled by the main loop
        for w in range(10):
            nc.tensor.matmul(
                wps[:, :256],
                wsrc[:, :, :128],
                wsrc[:],
                start=True,
                stop=True,
                perf_mode=DR,
            )

        for g, (c0, gs) in enumerate(GROUPS):
            gw = gs * NJ
            for mt in range(MT):
                psb = pp.tile([P, 4 * NJ], F32, tag="ps")
                sbh = []
                for s in range(gs):
                    cchunk = c0 + s
                    for t in range(KT // 2):
                        nc.tensor.matmul(
                            psb[:, s * NJ : (s + 1) * NJ],
                            xTl[:, mt, 2 * t : 2 * t + 2],
                            xT[:, cchunk, 2 * t : 2 * t + 2],
                            start=(t == 0),
                            stop=(t == KT // 2 - 1),
                            perf_mode=DR,
                        )
                    if s % 2 == 1:  # drain a completed 1024-wide half
                        sb = work.tile([P, 2 * NJ], F32, tag="sbh")
                        nc.scalar.copy(sb[:], psb[:, (s - 1) * NJ : (s + 1) * NJ])
                        sbh.append(sb)
                tmp = work.tile([P, NJ], F32, tag="tmp")
                if gs == 2:
                    nc.vector.tensor_max(tmp[:], sbh[0][:, :NJ], sbh[0][:, NJ:])
                else:
                    f1 = work.tile([P, 2 * NJ], F32, tag="f1")
                    nc.vector.tensor_max(f1[:], sbh[0][:], sbh[1][:])
                    nc.vector.tensor_max(tmp[:], f1[:, :NJ], f1[:, NJ:])
                nc.vector.max(out=cv[:, g, mt], in_=tmp[:])
                nc.vector.max_index(
                    out=ci[:, g, mt], in_max=cv[:, g, mt], in_values=tmp[:]
                )
            nc.sync.dma_start(out=cv_ap[:, g], in_=cv[:, g])
            nc.sync.dma_start(out=ci_ap[:, g], in_=ci[:, g])


def build_bass():
    nc = bacc.Bacc(
        "TRN2",
        target_bir_lowering=False,
        debug=False,
        enable_asserts=True,
        num_devices=NCORES,
    )
    x_t = nc.dram_tensor("xq", [P, CH, KT, CB], FP8, kind="ExternalInput").ap()
    xl_t = nc.dram_tensor("xql", [P, MT, KT, P], FP8, kind="ExternalInput").ap()
    cv_t = nc.dram_tensor("candv", [P, NG, MT, 8], F32, kind="ExternalOutput").ap()
    ci_t = nc.dram_tensor("candi", [P, NG, MT, 8], U16, kind="ExternalOutput").ap()
    with tile.TileContext(nc) as tc:
        emit_kernel(tc, x_t, xl_t, cv_t, ci_t)
    nc.compile()
    return nc


_XH = None  # host-side normalized input, set by make_in_maps


def make_in_maps(x: np.ndarray):
    global _XH
    norm = np.linalg.norm(x, axis=-1, keepdims=True)
    xh = (x / np.maximum(norm, EPS)).astype(np.float32)
    _XH = xh
    q8 = (xh * S).astype(ml_dtypes.float8_e4m3)
    # transposed: element [k, p, r] = q8[r, k*128 + p]; then chunk-contiguous
    # [P, CH, KT, CB] with [p, c, k, b] = q8[c*CB + b, k*128 + p]
    xt = q8.reshape(B, KT, P).transpose(1, 2, 0)  # [KT, P, B]
    xq = np.ascontiguousarray(
        xt.reshape(KT, P, CH, CB).transpose(1, 2, 0, 3)
    )  # [P, CH, KT, CB]
    ins = []
    for c in range(NCORES):
        # local stationary, mt-major: [P, MT, KT, 128]
        loc = xt[:, :, c * LOCAL : (c + 1) * LOCAL]  # [KT, P, LOCAL]
        xql = np.ascontiguousarray(
            loc.reshape(KT, P, MT, P).transpose(1, 2, 0, 3)
        )  # [P, MT, KT, 128]
        ins.append({"xq": xq, "xql": xql})
    return ins


def reduce_outputs(results):
    xh = _XH
    NC = NG * 8  # needles per row
    g_start = np.array([c0 * CB for c0, _ in GROUPS], dtype=np.int64)
    g_size = np.array([gs for _, gs in GROUPS], dtype=np.int64)
    allv = np.empty((B, NC), dtype=np.float32)
    allc = np.empty((B, NC), dtype=np.int64)  # column within fold (0..511)
    allg = np.empty((B, NC), dtype=np.int64)  # group id
    gids = np.broadcast_to(np.arange(NG)[None, :, None, None], (P, NG, MT, 8))
    for c, r in enumerate(results):
        v = np.asarray(r["candv"])  # [P, NG, MT, 8]
        ci = np.asarray(r["candi"]).astype(np.int64)
        sl = slice(c * LOCAL, (c + 1) * LOCAL)
        # row within core = mt*128 + p  ->  axes (mt, p, g, 8)
        allv[sl] = v.transpose(2, 0, 1, 3).reshape(LOCAL, NC)
        allc[sl] = ci.transpose(2, 0, 1, 3).reshape(LOCAL, NC)
        allg[sl] = gids.transpose(2, 0, 1, 3).reshape(LOCAL, NC)
    # top-8 needles by device value (true NN is always the top-1 needle)
    K = 8
    topk = np.argpartition(-allv, K, axis=-1)[:, :K]
    nc_ = np.take_along_axis(allc, topk, axis=-1)  # [B, K]
    ng_ = np.take_along_axis(allg, topk, axis=-1)
    # expand per-group subtile ambiguity: j = group_start + s*512 + c
    ss = np.arange(4)[None, None, :]
    cand = (
        g_start[ng_][:, :, None] + (ss % g_size[ng_][:, :, None]) * NJ + nc_[:, :, None]
    ).reshape(B, K * 4)
    rows = np.arange(B)[:, None]
    cos = np.einsum("rd,rkd->rk", xh, xh[cand], optimize=True)
    cos = np.where(cand == rows, -2.0, cos)  # exclude self-match
    jstar = cand[rows[:, 0], np.argmax(cos, axis=-1)]
    diff = xh - xh[jstar] + EPS
    dist = np.sqrt(np.sum(diff * diff, axis=-1))
    return np.mean(-np.log(dist + EPS)).astype(np.float32)


_LAST_RESULTS = None  # BassKernelResults of the most recent run (for test.py)


def run(x: np.ndarray, trace: bool = False):
    global _LAST_RESULTS
    nc = build_bass()
    res = bass_utils.run_bass_kernel_spmd(
        nc,
        make_in_maps(x),
        core_ids=list(range(NCORES)),
        trace=trace,
        trace_cores=list(range(NCORES)) if trace else None,
    )
    _LAST_RESULTS = res
    return reduce_outputs(res.results)


def kernel(**inputs) -> np.ndarray:
    x = np.asarray(inputs["student_output"], dtype=np.float32)
    assert x.shape == (B, D), x.shape
    return run(x, trace=False)


if __name__ == "__main__":
    rng = np.random.default_rng(0)
    x = rng.standard_normal((B, D), dtype=np.float32)
    print(kernel(student_output=x))


# revision 21
# speedup vs baseline: 1.0181x; 1.0181x over previous
"""KoLeo loss kernel for Trainium2 (8 NeuronCores, Bass/Tile).

reference semantics:
    x = student_output / max(||row||_2, 1e-8)        # [B, D] row-normalize
    dots = x @ x.T ; dots[i,i] = -1
    nn = argmax(dots, axis=1)
    d_i = || x_i - x_nn(i) + 1e-8 ||_2
    loss = mean(-log(d_i + 1e-8))

Device strategy (data-parallel over rows, 8 cores, identical NEFF):
  * Host normalizes rows in f32, scales by S=64 and quantizes to fp8 e4m3
    (TRN float8e4).  Each core receives the full quantized matrix in a
    transposed, chunk-contiguous layout plus its own 1024-row slice.
  * Device computes the row-sharded Gram G = q_local @ q_all.T with
    DoubleRow fp8 matmuls (K=256 per instruction, 2x PE throughput).
    Column tiles are processed in groups (two ramp groups of 1024
    columns while the input DMAs stream, then 2048-column groups),
    each accumulating into one multi-bank PSUM tile.
  * The Scalar (ACT) engine drains PSUM to SBUF in 1024-wide copies
    (the first half overlaps the group's remaining matmuls); DVE folds
    the group columnwise with a max tree down to [128, 512] (the row's
    true NN always survives: it is the row maximum, so it wins its
    column slot; GpSimd has no max ALU), then extracts the top-8 values
    + column indices (max / max_index).
  * Input DMAs are spread across the Sync/GpSimd/Scalar rings in
    consumption order; a short burst of warmup matmuls on a const tile
    flips the PE HAM clock gate to 8/8 while the first chunks stream.
  * Host merges the per-group needles, takes the global top-8 by device
    value per row (the true NN is always the top-1 needle), expands the
    per-group column ambiguity, recomputes exact f32 cosines for those
    candidates, drops the self-match, picks the true argmax and
    evaluates the reference loss formula exactly.
"""

import numpy as np
import ml_dtypes

import concourse.bacc as bacc
import concourse.bass as bass
import concourse.mybir as mybir
import concourse.tile as tile
from concourse import bass_utils

B, D, P = 8192, 1024, 128
NCORES = 8
LOCAL = B // NCORES  # 1024 rows per core
KT = D // P          # 8 contraction tiles of 128
MT = LOCAL // P      # 8 local row tiles
NJ = 512             # moving free dim per matmul
JT = B // NJ         # 16 column tiles
CH = 16              # input column chunks (one per column tile)
CB = B // CH         # 512 columns per chunk
GROUPS = [(0, 2), (2, 2), (4, 4), (8, 4), (12, 4)]  # (first chunk, #chunks)
NG = len(GROUPS)
S = 64.0             # fp8 pre-scale for normalized rows
EPS = 1e-8

F32 = mybir.dt.float32
FP8 = mybir.dt.float8e4
U16 = mybir.dt.uint16
DR = mybir.MatmulPerfMode.DoubleRow


def emit_kernel(tc, x_ap, xl_ap, cv_ap, ci_ap):
    nc = tc.nc
    with (
        tc.tile_pool(name="big", bufs=1) as big,
        tc.tile_pool(name="work", bufs=8) as work,
        tc.tile_pool(name="ps", bufs=2, space="PSUM") as pp,
    ):
        xT = big.tile([P, CH, KT, CB], FP8)
        xTl = big.tile([P, MT, KT, P], FP8)
        cv = big.tile([P, NG, MT, 8], F32)
        ci = big.tile([P, NG, MT, 8], U16)

        # input DMAs in consumption order, spread over the three DMA
        # rings; the Scalar ring gets only two early chunks so ACT
        # drains never queue behind DMA issues
        def ldx(c, eng):
            eng.dma_start(out=xT[:, c], in_=x_ap[:, c])

        nc.sync.dma_start(out=xTl[:, 0], in_=xl_ap[:, 0])
        ldx(0, nc.scalar)
        ldx(1, nc.gpsimd)
        for mt in range(1, MT):
            nc.sync.dma_start(out=xTl[:, mt], in_=xl_ap[:, mt])
        ldx(3, nc.scalar)
        ldx(4, nc.gpsimd)
        ldx(2, nc.sync)
        for c in range(5, CH):
            ldx(c, nc.gpsimd if c % 2 == 0 else nc.sync)

        # PE warmup on a resident const tile while the first chunks
        # stream, so the HAM clock gate is at 8/8 for the real matmuls
        wsrc = big.tile([P, 2, 256], FP8)
        nc.vector.memset(wsrc[:], 0.25)
        wps = pp.tile([P, 4 * NJ], F32, tag="ps")  # recycled by the main loop
        for w in range(10):
            nc.tensor.matmul(
                wps[:, :256],
                wsrc[:, :, :128],
                wsrc[:],
                start=True,
                stop=True,
                perf_mode=DR,
            )

        for g, (c0, gs) in enumerate(GROUPS):
            gw = gs * NJ
            for mt in range(MT):
                psb = pp.tile([P, 4 * NJ], F32, tag="ps")
                sbh = []
                for s in range(gs):
                    cchunk = c0 + s
                    for t in range(KT // 2):
                        nc.tensor.matmul(
                            psb[:, s * NJ : (s + 1) * NJ],
                            xTl[:, mt, 2 * t : 2 * t + 2],
                            xT[:, cchunk, 2 * t : 2 * t + 2],
                            start=(t == 0),
                            stop=(t == KT // 2 - 1),
                            perf_mode=DR,
                        )
                    if s % 2 == 1:  # drain a completed 1024-wide half
                        sb = work.tile([P, 2 * NJ], F32, tag="sbh")
                        nc.scalar.copy(sb[:], psb[:, (s - 1) * NJ : (s + 1) * NJ])
                        sbh.append(sb)
                tmp = work.tile([P, NJ], F32, tag="tmp")
                if gs == 2:
                    nc.vector.tensor_max(tmp[:], sbh[0][:, :NJ], sbh[0][:, NJ:])
                else:
                    f1 = work.tile([P, 2 * NJ], F32, tag="f1")
                    nc.vector.tensor_max(f1[:], sbh[0][:], sbh[1][:])
                    nc.vector.tensor_max(tmp[:], f1[:, :NJ], f1[:, NJ:])
                nc.vector.max(out=cv[:, g, mt], in_=tmp[:])
                nc.vector.max_index(
                    out=ci[:, g, mt], in_max=cv[:, g, mt], in_values=tmp[:]
                )
            nc.sync.dma_start(out=cv_ap[:, g], in_=cv[:, g])
            nc.sync.dma_start(out=ci_ap[:, g], in_=ci[:, g])


def build_bass():
    nc = bacc.Bacc(
        "TRN2",
        target_bir_lowering=False,
        debug=False,
        enable_asserts=True,
        num_devices=NCORES,
    )
    x_t = nc.dram_tensor("xq", [P, CH, KT, CB], FP8, kind="ExternalInput").ap()
    xl_t = nc.dram_tensor("xql", [P, MT, KT, P], FP8, kind="ExternalInput").ap()
    cv_t = nc.dram_tensor("candv", [P, NG, MT, 8], F32, kind="ExternalOutput").ap()
    ci_t = nc.dram_tensor("candi", [P, NG, MT, 8], U16, kind="ExternalOutput").ap()
    with tile.TileContext(nc) as tc:
        emit_kernel(tc, x_t, xl_t, cv_t, ci_t)
    nc.compile()
    return nc


_XH = None  # host-side normalized input, set by make_in_maps


def make_in_maps(x: np.ndarray):
    global _XH
    norm = np.linalg.norm(x, axis=-1, keepdims=True)
    xh = (x / np.maximum(norm, EPS)).astype(np.float32)
    _XH = xh
    q8 = (xh * S).astype(ml_dtypes.float8_e4m3)
    # transposed: element [k, p, r] = q8[r, k*128 + p]; then chunk-contiguous
    # [P, CH, KT, CB] with [p, c, k, b] = q8[c*CB + b, k*128 + p]
    xt = q8.reshape(B, KT, P).transpose(1, 2, 0)  # [KT, P, B]
    xq = np.ascontiguousarray(
        xt.reshape(KT, P, CH, CB).transpose(1, 2, 0, 3)
    )  # [P, CH, KT, CB]
    ins = []
    for c in range(NCORES):
        # local stationary, mt-major: [P, MT, KT, 128]
        loc = xt[:, :, c * LOCAL : (c + 1) * LOCAL]  # [KT, P, LOCAL]
        xql = np.ascontiguousarray(
            loc.reshape(KT, P, MT, P).transpose(1, 2, 0, 3)
        )  # [P, MT, KT, 128]
        ins.append({"xq": xq, "xql": xql})
    return ins


def reduce_outputs(results):
    xh = _XH
    NC = NG * 8  # needles per row
    g_start = np.array([c0 * CB for c0, _ in GROUPS], dtype=np.int64)
    g_size = np.array([gs for _, gs in GROUPS], dtype=np.int64)
    allv = np.empty((B, NC), dtype=np.float32)
    allc = np.empty((B, NC), dtype=np.int64)  # column within fold (0..511)
    allg = np.empty((B, NC), dtype=np.int64)  # group id
    gids = np.broadcast_to(np.arange(NG)[None, :, None, None], (P, NG, MT, 8))
    for c, r in enumerate(results):
        v = np.asarray(r["candv"])  # [P, NG, MT, 8]
        ci = np.asarray(r["candi"]).astype(np.int64)
        sl = slice(c * LOCAL, (c + 1) * LOCAL)
        # row within core = mt*128 + p  ->  axes (mt, p, g, 8)
        allv[sl] = v.transpose(2, 0, 1, 3).reshape(LOCAL, NC)
        allc[sl] = ci.transpose(2, 0, 1, 3).reshape(LOCAL, NC)
        allg[sl] = gids.transpose(2, 0, 1, 3).reshape(LOCAL, NC)
    # top-8 needles by device value (true NN is always the top-1 needle)
    K = 8
    topk = np.argpartition(-allv, K, axis=-1)[:, :K]
    nc_ = np.take_along_axis(allc, topk, axis=-1)  # [B, K]
    ng_ = np.take_along_axis(allg, topk, axis=-1)
    # expand per-group subtile ambiguity: j = group_start + s*512 + c
    ss = np.arange(4)[None, None, :]
    cand = (
        g_start[ng_][:, :, None] + (ss % g_size[ng_][:, :, None]) * NJ + nc_[:, :, None]
    ).reshape(B, K * 4)
    rows = np.arange(B)[:, None]
    cos = np.einsum("rd,rkd->rk", xh, xh[cand], optimize=True)
    cos = np.where(cand == rows, -2.0, cos)  # exclude self-match
    jstar = cand[rows[:, 0], np.argmax(cos, axis=-1)]
    diff = xh - xh[jstar] + EPS
    dist = np.sqrt(np.sum(diff * diff, axis=-1))
    return np.mean(-np.log(dist + EPS)).astype(np.float32)


_LAST_RESULTS = None  # BassKernelResults of the most recent run (for test.py)


def run(x: np.ndarray, trace: bool = False):
    global _LAST_RESULTS
    nc = build_bass()
    res = bass_utils.run_bass_kernel_spmd(
        nc,
        make_in_maps(x),
        core_ids=list(range(NCORES)),
        trace=trace,
        trace_cores=list(range(NCORES)) if trace else None,
    )
    _LAST_RESULTS = res
    return reduce_outputs(res.results)


def kernel(**inputs) -> np.ndarray:
    x = np.asarray(inputs["student_output"], dtype=np.float32)
    assert x.shape == (B, D), x.shape
    return run(x, trace=False)


if __name__ == "__main__":
    rng = np.random.default_rng(0)
    x = rng.standard_normal((B, D), dtype=np.float32)
    print(kernel(student_output=x))


# revision 24
# speedup vs baseline: 1.2624x; 1.2400x over previous
"""KoLeo loss kernel for Trainium2 (8 NeuronCores, Bass/Tile).

reference semantics:
    x = student_output / max(||row||_2, 1e-8)        # [B, D] row-normalize
    dots = x @ x.T ; dots[i,i] = -1
    nn = argmax(dots, axis=1)
    d_i = || x_i - x_nn(i) + 1e-8 ||_2
    loss = mean(-log(d_i + 1e-8))

Device strategy (data-parallel over rows, 8 cores, identical NEFF):
  * Host normalizes rows in f32, scales by S=64 and quantizes to fp8 e4m3
    (TRN float8e4).  Each core receives the full quantized matrix in a
    transposed, chunk-contiguous layout plus its own 1024-row slice.
  * Device computes the row-sharded Gram G = q_local @ q_all.T with
    DoubleRow fp8 matmuls (K=256 per instruction, 2x PE throughput).
    Column tiles are processed in groups (two ramp groups of 1024
    columns while the input DMAs stream, then 2048-column groups),
    each accumulating into one multi-bank PSUM tile.
  * The Scalar (ACT) engine drains PSUM to SBUF in 1024-wide copies
    (the first half overlaps the group's remaining matmuls); DVE folds
    the group columnwise with a max tree down to [128, 512] (the row's
    true NN always survives: it is the row maximum, so it wins its
    column slot; GpSimd has no max ALU), then extracts the top-8 values
    + column indices (max / max_index).
  * Input DMAs are spread across the Sync/GpSimd/Scalar rings in
    consumption order; a short burst of warmup matmuls on a const tile
    flips the PE HAM clock gate to 8/8 while the first chunks stream.
  * Host merges the per-group needles, takes the global top-8 by device
    value per row (the true NN is always the top-1 needle), expands the
    per-group column ambiguity, recomputes exact f32 cosines for those
    candidates, drops the self-match, picks the true argmax and
    evaluates the reference loss formula exactly.
"""

import numpy as np
import ml_dtypes

import concourse.bacc as bacc
import concourse.bass as bass
import concourse.mybir as mybir
import concourse.tile as tile
from concourse import bass_utils

B, D, P = 8192, 1024, 128
NCORES = 8
LOCAL = B // NCORES  # 1024 rows per core
KT = D // P          # 8 contraction tiles of 128
MT = LOCAL // P      # 8 local row tiles
NJ = 512             # moving free dim per matmul
JT = B // NJ         # 16 column tiles
CH = 16              # input column chunks (one per column tile)
CB = B // CH         # 512 columns per chunk
GROUPS = [(0, 2), (2, 2), (4, 4), (8, 4), (12, 4)]  # (first chunk, #chunks)
NG = len(GROUPS)
S = 64.0             # fp8 pre-scale for normalized rows
EPS = 1e-8

F32 = mybir.dt.float32
FP8 = mybir.dt.float8e4
U16 = mybir.dt.uint16
DR = mybir.MatmulPerfMode.DoubleRow


def emit_kernel(tc, x_ap, xl_ap, cv_ap, ci_ap):
    nc = tc.nc
    with (
        tc.tile_pool(name="big", bufs=1) as big,
        tc.tile_pool(name="work", bufs=6) as work,
        tc.tile_pool(name="ps", bufs=4, space="PSUM") as pp,
    ):
        xT = big.tile([P, CH, KT, CB], FP8)
        xTl = big.tile([P, MT, KT, P], FP8)
        cv = big.tile([P, NG, MT, 8], F32)
        ci = big.tile([P, NG, MT, 8], U16)

        # input DMAs in consumption order, spread over the three DMA
        # rings; the Scalar ring gets only two early chunks so ACT
        # drains never queue behind DMA issues
        def ldx(c, eng):
            eng.dma_start(out=xT[:, c], in_=x_ap[:, c])

        nc.sync.dma_start(out=xTl[:, 0], in_=xl_ap[:, 0])
        ldx(0, nc.scalar)
        ldx(1, nc.gpsimd)
        for mt in range(1, MT):
            nc.sync.dma_start(out=xTl[:, mt], in_=xl_ap[:, mt])
        ldx(3, nc.scalar)
        ldx(4, nc.gpsimd)
        ldx(2, nc.sync)
        for c in range(5, CH):
            ldx(c, nc.gpsimd if c % 2 == 0 else nc.sync)

        # PE warmup on a resident const tile while the first chunks
        # stream, so the HAM clock gate is at 8/8 for the real matmuls
        wsrc = big.tile([P, 2, 256], FP8)
        nc.vector.memset(wsrc[:], 0.25)
        wps = pp.tile([P, 2 * NJ], F32, tag="ps")  # recycled by the main loop
        for w in range(10):
            nc.tensor.matmul(
                wps[:, :256],
                wsrc[:, :, :128],
                wsrc[:],
                start=True,
                stop=True,
                perf_mode=DR,
            )

        for g, (c0, gs) in enumerate(GROUPS):
            for mt in range(MT):
                sbh = []
                for half in range(gs // 2):
                    psb = pp.tile([P, 2 * NJ], F32, tag="ps")
                    for s2 in range(2):
                        s = 2 * half + s2
                        cchunk = c0 + s
                        for t in range(KT // 2):
                            nc.tensor.matmul(
                                psb[:, s2 * NJ : (s2 + 1) * NJ],
                                xTl[:, mt, 2 * t : 2 * t + 2],
                                xT[:, cchunk, 2 * t : 2 * t + 2],
                                start=(t == 0),
                                stop=(t == KT // 2 - 1),
                                perf_mode=DR,
                            )
                    sb = work.tile([P, 2 * NJ], F32, tag="sbh")
                    nc.scalar.copy(sb[:], psb[:])  # ACT drains the 2-bank tile
                    sbh.append(sb)
                tmp = work.tile([P, NJ], F32, tag="tmp")
                if gs == 2:
                    nc.vector.tensor_max(tmp[:], sbh[0][:, :NJ], sbh[0][:, NJ:])
                else:
                    f1 = work.tile([P, 2 * NJ], F32, tag="f1")
                    nc.vector.tensor_max(f1[:], sbh[0][:], sbh[1][:])
                    nc.vector.tensor_max(tmp[:], f1[:, :NJ], f1[:, NJ:])
                nc.vector.max(out=cv[:, g, mt], in_=tmp[:])
                nc.vector.max_index(
                    out=ci[:, g, mt], in_max=cv[:, g, mt], in_values=tmp[:]
                )
            nc.sync.dma_start(out=cv_ap[:, g], in_=cv[:, g])
            nc.sync.dma_start(out=ci_ap[:, g], in_=ci[:, g])


def build_bass():
    nc = bacc.Bacc(
        "TRN2",
        target_bir_lowering=False,
        debug=False,
        enable_asserts=True,
        num_devices=NCORES,
    )
    x_t = nc.dram_tensor("xq", [P, CH, KT, CB], FP8, kind="ExternalInput").ap()
    xl_t = nc.dram_tensor("xql", [P, MT, KT, P], FP8, kind="ExternalInput").ap()
    cv_t = nc.dram_tensor("candv", [P, NG, MT, 8], F32, kind="ExternalOutput").ap()
    ci_t = nc.dram_tensor("candi", [P, NG, MT, 8], U16, kind="ExternalOutput").ap()
    with tile.TileContext(nc) as tc:
        emit_kernel(tc, x_t, xl_t, cv_t, ci_t)
    nc.compile()
    return nc


_XH = None  # host-side normalized input, set by make_in_maps


def make_in_maps(x: np.ndarray):
    global _XH
    norm = np.linalg.norm(x, axis=-1, keepdims=True)
    xh = (x / np.maximum(norm, EPS)).astype(np.float32)
    _XH = xh
    q8 = (xh * S).astype(ml_dtypes.float8_e4m3)
    # transposed: element [k, p, r] = q8[r, k*128 + p]; then chunk-contiguous
    # [P, CH, KT, CB] with [p, c, k, b] = q8[c*CB + b, k*128 + p]
    xt = q8.reshape(B, KT, P).transpose(1, 2, 0)  # [KT, P, B]
    xq = np.ascontiguousarray(
        xt.reshape(KT, P, CH, CB).transpose(1, 2, 0, 3)
    )  # [P, CH, KT, CB]
    ins = []
    for c in range(NCORES):
        # local stationary, mt-major: [P, MT, KT, 128]
        loc = xt[:, :, c * LOCAL : (c + 1) * LOCAL]  # [KT, P, LOCAL]
        xql = np.ascontiguousarray(
            loc.reshape(KT, P, MT, P).transpose(1, 2, 0, 3)
        )  # [P, MT, KT, 128]
        ins.append({"xq": xq, "xql": xql})
    return ins


def reduce_outputs(results):
    xh = _XH
    NC = NG * 8  # needles per row
    g_start = np.array([c0 * CB for c0, _ in GROUPS], dtype=np.int64)
    g_size = np.array([gs for _, gs in GROUPS], dtype=np.int64)
    allv = np.empty((B, NC), dtype=np.float32)
    allc = np.empty((B, NC), dtype=np.int64)  # column within fold (0..511)
    allg = np.empty((B, NC), dtype=np.int64)  # group id
    gids = np.broadcast_to(np.arange(NG)[None, :, None, None], (P, NG, MT, 8))
    for c, r in enumerate(results):
        v = np.asarray(r["candv"])  # [P, NG, MT, 8]
        ci = np.asarray(r["candi"]).astype(np.int64)
        sl = slice(c * LOCAL, (c + 1) * LOCAL)
        # row within core = mt*128 + p  ->  axes (mt, p, g, 8)
        allv[sl] = v.transpose(2, 0, 1, 3).reshape(LOCAL, NC)
        allc[sl] = ci.transpose(2, 0, 1, 3).reshape(LOCAL, NC)
        allg[sl] = gids.transpose(2, 0, 1, 3).reshape(LOCAL, NC)
    # top-8 needles by device value (true NN is always the top-1 needle)
    K = 8
    topk = np.argpartition(-allv, K, axis=-1)[:, :K]
    nc_ = np.take_along_axis(allc, topk, axis=-1)  # [B, K]
    ng_ = np.take_along_axis(allg, topk, axis=-1)
    # expand per-group subtile ambiguity: j = group_start + s*512 + c
    ss = np.arange(4)[None, None, :]
    cand = (
        g_start[ng_][:, :, None] + (ss % g_size[ng_][:, :, None]) * NJ + nc_[:, :, None]
    ).reshape(B, K * 4)
    rows = np.arange(B)[:, None]
    cos = np.einsum("rd,rkd->rk", xh, xh[cand], optimize=True)
    cos = np.where(cand == rows, -2.0, cos)  # exclude self-match
    jstar = cand[rows[:, 0], np.argmax(cos, axis=-1)]
    diff = xh - xh[jstar] + EPS
    dist = np.sqrt(np.sum(diff * diff, axis=-1))
    return np.mean(-np.log(dist + EPS)).astype(np.float32)


_LAST_RESULTS = None  # BassKernelResults of the most recent run (for test.py)


def run(x: np.ndarray, trace: bool = False):
    global _LAST_RESULTS
    nc = build_bass()
    res = bass_utils.run_bass_kernel_spmd(
        nc,
        make_in_maps(x),
        core_ids=list(range(NCORES)),
        trace=trace,
        trace_cores=list(range(NCORES)) if trace else None,
    )
    _LAST_RESULTS = res
    return reduce_outputs(res.results)


def kernel(**inputs) -> np.ndarray:
    x = np.asarray(inputs["student_output"], dtype=np.float32)
    assert x.shape == (B, D), x.shape
    return run(x, trace=False)


if __name__ == "__main__":
    rng = np.random.default_rng(0)
    x = rng.standard_normal((B, D), dtype=np.float32)
    print(kernel(student_output=x))


# revision 26
# speedup vs baseline: 1.2850x; 1.0179x over previous
"""KoLeo loss kernel for Trainium2 (8 NeuronCores, Bass/Tile).

reference semantics:
    x = student_output / max(||row||_2, 1e-8)        # [B, D] row-normalize
    dots = x @ x.T ; dots[i,i] = -1
    nn = argmax(dots, axis=1)
    d_i = || x_i - x_nn(i) + 1e-8 ||_2
    loss = mean(-log(d_i + 1e-8))

Device strategy (data-parallel over rows, 8 cores, identical NEFF):
  * Host normalizes rows in f32, scales by S=64 and quantizes to fp8 e4m3
    (TRN float8e4).  Each core receives the full quantized matrix in a
    transposed, chunk-contiguous layout plus its own 1024-row slice.
  * Device computes the row-sharded Gram G = q_local @ q_all.T with
    DoubleRow fp8 matmuls (K=256 per instruction, 2x PE throughput).
    Column tiles are processed in groups (two ramp groups of 1024
    columns while the input DMAs stream, then 2048-column groups),
    each accumulating into one multi-bank PSUM tile.
  * The Scalar (ACT) engine drains PSUM to SBUF in 1024-wide copies
    (the first half overlaps the group's remaining matmuls); DVE folds
    the group columnwise with a max tree down to [128, 512] (the row's
    true NN always survives: it is the row maximum, so it wins its
    column slot; GpSimd has no max ALU), then extracts the top-8 values
    + column indices (max / max_index).
  * Input DMAs are spread across the Sync/GpSimd/Scalar rings in
    consumption order; a short burst of warmup matmuls on a const tile
    flips the PE HAM clock gate to 8/8 while the first chunks stream.
  * Host merges the per-group needles, takes the global top-8 by device
    value per row (the true NN is always the top-1 needle), expands the
    per-group column ambiguity, recomputes exact f32 cosines for those
    candidates, drops the self-match, picks the true argmax and
    evaluates the reference loss formula exactly.
"""

import numpy as np
import ml_dtypes

import concourse.bacc as bacc
import concourse.bass as bass
import concourse.mybir as mybir
import concourse.tile as tile
from concourse import bass_utils

B, D, P = 8192, 1024, 128
NCORES = 8
LOCAL = B // NCORES  # 1024 rows per core
KT = D // P          # 8 contraction tiles of 128
MT = LOCAL // P      # 8 local row tiles
NJ = 512             # moving free dim per matmul
JT = B // NJ         # 16 column tiles
CH = 16              # input column chunks (one per column tile)
CB = B // CH         # 512 columns per chunk
GROUPS = [(0, 2), (2, 2), (4, 4), (8, 4), (12, 4)]  # (first chunk, #chunks)
NG = len(GROUPS)
S = 64.0             # fp8 pre-scale for normalized rows
EPS = 1e-8

F32 = mybir.dt.float32
FP8 = mybir.dt.float8e4
U16 = mybir.dt.uint16
DR = mybir.MatmulPerfMode.DoubleRow


def emit_kernel(tc, x_ap, xl_ap, cv_ap, ci_ap):
    nc = tc.nc
    with (
        tc.tile_pool(name="big", bufs=1) as big,
        tc.tile_pool(name="work", bufs=6) as work,
        tc.tile_pool(name="ps", bufs=4, space="PSUM") as pp,
    ):
        xT = big.tile([P, CH, KT, CB], FP8)
        xTl = big.tile([P, MT, KT, P], FP8)
        cv = big.tile([P, NG, MT, 8], F32)
        ci = big.tile([P, NG, MT, 8], U16)

        # input DMAs in consumption order, spread over the three DMA
        # rings; the Scalar ring gets only two early chunks so ACT
        # drains never queue behind DMA issues
        def ldx(c, eng):
            eng.dma_start(out=xT[:, c], in_=x_ap[:, c])

        nc.sync.dma_start(out=xTl[:, 0], in_=xl_ap[:, 0])
        ldx(0, nc.scalar)
        ldx(1, nc.gpsimd)
        for mt in range(1, MT):
            nc.sync.dma_start(out=xTl[:, mt], in_=xl_ap[:, mt])
        ldx(3, nc.scalar)
        ldx(4, nc.gpsimd)
        ldx(2, nc.sync)
        for c in range(5, CH):
            ldx(c, nc.gpsimd if c % 2 == 0 else nc.sync)

        # PE warmup on a resident const tile while the first chunks
        # stream, so the HAM clock gate is at 8/8 for the real matmuls
        wsrc = big.tile([P, 2, 256], FP8)
        nc.vector.memset(wsrc[:], 0.25)
        wps = pp.tile([P, 2 * NJ], F32, tag="ps")  # recycled by the main loop
        for w in range(30):
            nc.tensor.matmul(
                wps[:, :256],
                wsrc[:, :, :128],
                wsrc[:],
                start=True,
                stop=True,
                perf_mode=DR,
            )

        for g, (c0, gs) in enumerate(GROUPS):
            for mt in range(MT):
                last = g == NG - 1 and mt == MT - 1
                sbh = []
                psbs = []
                for half in range(gs // 2):
                    psb = pp.tile([P, 2 * NJ], F32, tag="ps")
                    psbs.append(psb)
                    for s2 in range(2):
                        s = 2 * half + s2
                        cchunk = c0 + s
                        for t in range(KT // 2):
                            nc.tensor.matmul(
                                psb[:, s2 * NJ : (s2 + 1) * NJ],
                                xTl[:, mt, 2 * t : 2 * t + 2],
                                xT[:, cchunk, 2 * t : 2 * t + 2],
                                start=(t == 0),
                                stop=(t == KT // 2 - 1),
                                perf_mode=DR,
                            )
                    if last and half == gs // 2 - 1:
                        continue  # final half folds straight from PSUM
                    sb = work.tile([P, 2 * NJ], F32, tag="sbh")
                    nc.scalar.copy(sb[:], psb[:])  # ACT drains the 2-bank tile
                    sbh.append(sb)
                tmp = work.tile([P, NJ], F32, tag="tmp")
                if gs == 2:
                    nc.vector.tensor_max(tmp[:], sbh[0][:, :NJ], sbh[0][:, NJ:])
                elif last:
                    f1 = work.tile([P, 2 * NJ], F32, tag="f1")
                    nc.vector.tensor_max(f1[:], sbh[0][:], psbs[1][:])
                    nc.vector.tensor_max(tmp[:], f1[:, :NJ], f1[:, NJ:])
                else:
                    f1 = work.tile([P, 2 * NJ], F32, tag="f1")
                    nc.vector.tensor_max(f1[:], sbh[0][:], sbh[1][:])
                    nc.vector.tensor_max(tmp[:], f1[:, :NJ], f1[:, NJ:])
                nc.vector.max(out=cv[:, g, mt], in_=tmp[:])
                nc.vector.max_index(
                    out=ci[:, g, mt], in_max=cv[:, g, mt], in_values=tmp[:]
                )
            nc.sync.dma_start(out=cv_ap[:, g], in_=cv[:, g])
            nc.sync.dma_start(out=ci_ap[:, g], in_=ci[:, g])


def build_bass():
    nc = bacc.Bacc(
        "TRN2",
        target_bir_lowering=False,
        debug=False,
        enable_asserts=True,
        num_devices=NCORES,
    )
    x_t = nc.dram_tensor("xq", [P, CH, KT, CB], FP8, kind="ExternalInput").ap()
    xl_t = nc.dram_tensor("xql", [P, MT, KT, P], FP8, kind="ExternalInput").ap()
    cv_t = nc.dram_tensor("candv", [P, NG, MT, 8], F32, kind="ExternalOutput").ap()
    ci_t = nc.dram_tensor("candi", [P, NG, MT, 8], U16, kind="ExternalOutput").ap()
    with tile.TileContext(nc) as tc:
        emit_kernel(tc, x_t, xl_t, cv_t, ci_t)
    nc.compile()
    return nc


_XH = None  # host-side normalized input, set by make_in_maps


def make_in_maps(x: np.ndarray):
    global _XH
    norm = np.linalg.norm(x, axis=-1, keepdims=True)
    xh = (x / np.maximum(norm, EPS)).astype(np.float32)
    _XH = xh
    q8 = (xh * S).astype(ml_dtypes.float8_e4m3)
    # transposed: element [k, p, r] = q8[r, k*128 + p]; then chunk-contiguous
    # [P, CH, KT, CB] with [p, c, k, b] = q8[c*CB + b, k*128 + p]
    xt = q8.reshape(B, KT, P).transpose(1, 2, 0)  # [KT, P, B]
    xq = np.ascontiguousarray(
        xt.reshape(KT, P, CH, CB).transpose(1, 2, 0, 3)
    )  # [P, CH, KT, CB]
    ins = []
    for c in range(NCORES):
        # local stationary, mt-major: [P, MT, KT, 128]
        loc = xt[:, :, c * LOCAL : (c + 1) * LOCAL]  # [KT, P, LOCAL]
        xql = np.ascontiguousarray(
            loc.reshape(KT, P, MT, P).transpose(1, 2, 0, 3)
        )  # [P, MT, KT, 128]
        ins.append({"xq": xq, "xql": xql})
    return ins


def reduce_outputs(results):
    xh = _XH
    NC = NG * 8  # needles per row
    g_start = np.array([c0 * CB for c0, _ in GROUPS], dtype=np.int64)
    g_size = np.array([gs for _, gs in GROUPS], dtype=np.int64)
    allv = np.empty((B, NC), dtype=np.float32)
    allc = np.empty((B, NC), dtype=np.int64)  # column within fold (0..511)
    allg = np.empty((B, NC), dtype=np.int64)  # group id
    gids = np.broadcast_to(np.arange(NG)[None, :, None, None], (P, NG, MT, 8))
    for c, r in enumerate(results):
        v = np.asarray(r["candv"])  # [P, NG, MT, 8]
        ci = np.asarray(r["candi"]).astype(np.int64)
        sl = slice(c * LOCAL, (c + 1) * LOCAL)
        # row within core = mt*128 + p  ->  axes (mt, p, g, 8)
        allv[sl] = v.transpose(2, 0, 1, 3).reshape(LOCAL, NC)
        allc[sl] = ci.transpose(2, 0, 1, 3).reshape(LOCAL, NC)
        allg[sl] = gids.transpose(2, 0, 1, 3).reshape(LOCAL, NC)
    # top-8 needles by device value (true NN is always the top-1 needle)
    K = 8
    topk = np.argpartition(-allv, K, axis=-1)[:, :K]
    nc_ = np.take_along_axis(allc, topk, axis=-1)  # [B, K]
    ng_ = np.take_along_axis(allg, topk, axis=-1)
    # expand per-group subtile ambiguity: j = group_start + s*512 + c
    ss = np.arange(4)[None, None, :]
    cand = (
        g_start[ng_][:, :, None] + (ss % g_size[ng_][:, :, None]) * NJ + nc_[:, :, None]
    ).reshape(B, K * 4)
    rows = np.arange(B)[:, None]
    cos = np.einsum("rd,rkd->rk", xh, xh[cand], optimize=True)
    cos = np.where(cand == rows, -2.0, cos)  # exclude self-match
    jstar = cand[rows[:, 0], np.argmax(cos, axis=-1)]
    diff = xh - xh[jstar] + EPS
    dist = np.sqrt(np.sum(diff * diff, axis=-1))
    return np.mean(-np.log(dist + EPS)).astype(np.float32)


_LAST_RESULTS = None  # BassKernelResults of the most recent run (for test.py)


def run(x: np.ndarray, trace: bool = False):
    global _LAST_RESULTS
    nc = build_bass()
    res = bass_utils.run_bass_kernel_spmd(
        nc,
        make_in_maps(x),
        core_ids=list(range(NCORES)),
        trace=trace,
        trace_cores=list(range(NCORES)) if trace else None,
    )
    _LAST_RESULTS = res
    return reduce_outputs(res.results)


def kernel(**inputs) -> np.ndarray:
    x = np.asarray(inputs["student_output"], dtype=np.float32)
    assert x.shape == (B, D), x.shape
    return run(x, trace=False)


if __name__ == "__main__":
    rng = np.random.default_rng(0)
    x = rng.standard_normal((B, D), dtype=np.float32)
    print(kernel(student_output=x))
